# revision 6
# baseline (speedup 1.0000x reference)
"""3-layer GCN on 8 Trainium2 NeuronCores (Bass/Tile).

Math: with A = D^-1/2 (Adj + I) D^-1/2 (PyG GCNConv norm, self-loops),
each layer is h' = leaky_relu(A h W + b) = leaky_relu((A h) W + b).
Factor A h = dinv * ((Adj+I)(dinv * h)), so aggregation is an unweighted
gather-sum over in-edges of the row-scaled feature table u = dinv*(h@W).

Sharding: nodes are split 6250/core (8 cores). Per layer each core
computes its shard of the table u, an AllGather replicates the full
table [50176, 96] to every core's DRAM, then each core aggregates its
own targets with indirect-DMA row gathers over a host-built padded CSR
(targets degree-sorted per core so each 128-row block has near-uniform
degree). All index prep runs on host; all FLOPs on device.
"""
import os
import numpy as np
from contextlib import ExitStack

import concourse.bass as bass
import concourse.tile as tile
from concourse import bacc, mybir
from concourse.bass_utils import run_bass_kernel_spmd
from concourse.masks import make_identity

N = 50000
E = 800000
IN_F = 128
H = 96
C_OUT = 21
CORES = 8
SH = N // CORES          # 6250 real nodes per core
NBLK = 49                # ceil(6250/128)
SHP = NBLK * 128         # 6272 padded rows per shard
TBL = CORES * SHP        # 50176 table rows
SLOPE = 0.01

F32 = mybir.dt.float32
I32 = mybir.dt.int32

# stash for test.py introspection (exec time etc.)
LAST_RESULTS = None


def _host_prep(x, edge_index):
    """Build permutations, padded CSR (uniform per-block degree across cores),
    per-core transposed x, dinv blocks. Returns per-core input arrays and
    unpermute info."""
    src = np.asarray(edge_index[0], dtype=np.int64)
    tgt = np.asarray(edge_index[1], dtype=np.int64)
    deg = np.bincount(tgt, minlength=N).astype(np.float64) + 1.0
    dinv = (1.0 / np.sqrt(deg)).astype(np.float32)

    core_of = tgt // SH

    orders = []      # per core: position -> local node id
    poss = []        # per core: local node id -> position
    for c in range(CORES):
        indeg = deg[c * SH:(c + 1) * SH]
        order = np.argsort(-indeg, kind="stable")
        pos = np.empty(SH, dtype=np.int64)
        pos[order] = np.arange(SH)
        orders.append(order)
        poss.append(pos)

    # global node -> table row
    r_of = np.empty(N, dtype=np.int64)
    for c in range(CORES):
        r_of[c * SH:(c + 1) * SH] = c * SHP + poss[c]

    # per-core neighbor grids [SHP, Dmax_c] and lengths
    lens = np.zeros((CORES, SHP), dtype=np.int64)
    grids = []
    for c in range(CORES):
        sel = core_of == c
        s_c = src[sel]
        t_c = tgt[sel]
        pt = poss[c][t_c - c * SH]          # target position within shard
        o = np.argsort(pt, kind="stable")
        pt_s = pt[o]
        rows_src = r_of[s_c[o]].astype(np.int32)
        cnt = np.bincount(pt_s, minlength=SHP)
        lens[c] = cnt
        starts = np.zeros(SHP + 1, dtype=np.int64)
        np.cumsum(cnt, out=starts[1:])
        col = np.arange(len(pt_s)) - starts[pt_s]
        dmax = int(cnt.max()) if len(pt_s) else 0
        pad_row = np.int32(c * SHP + SH)    # a zeroed pad row of own shard
        grid = np.full((SHP, max(dmax, 1)), pad_row, dtype=np.int32)
        grid[pt_s, col] = rows_src
        grids.append(grid)

    # uniform per-block degree across cores
    D = lens.reshape(CORES, NBLK, 128).max(axis=(0, 2)).astype(np.int64)
    offs = np.zeros(NBLK + 1, dtype=np.int64)
    np.cumsum(D, out=offs[1:])
    sumd = int(offs[-1])

    nbrs = []
    for c in range(CORES):
        pad_row = np.int32(c * SHP + SH)
        nbr = np.full((128, sumd), pad_row, dtype=np.int32)
        g = grids[c]
        for b in range(NBLK):
            d = int(D[b])
            if d == 0:
                continue
            blk = g[b * 128:(b + 1) * 128, :min(d, g.shape[1])]
            nbr[:, offs[b]:offs[b] + blk.shape[1]] = blk
        nbrs.append(nbr)

    xTs, dinv_blks = [], []
    for c in range(CORES):
        xs = np.zeros((SHP, IN_F), dtype=np.float32)
        xs[:SH] = np.asarray(x[c * SH:(c + 1) * SH], dtype=np.float32)[orders[c]]
        xTs.append(np.ascontiguousarray(xs.T))
        db = np.ones(SHP, dtype=np.float32)
        db[:SH] = dinv[c * SH:(c + 1) * SH][orders[c]]
        dinv_blks.append(np.ascontiguousarray(db.reshape(NBLK, 128).T))
    return xTs, nbrs, dinv_blks, [int(d) for d in D], [int(o) for o in offs], poss


def _build_bass(D, offs, sumd):
    nc = bacc.Bacc("TRN2", target_bir_lowering=False, debug=False,
                   num_devices=CORES)

    xT_in = nc.declare_dram_parameter("xT", [IN_F, SHP], F32, isOutput=False)
    nbr_in = nc.declare_dram_parameter("nbr", [128, max(sumd, 1)], I32, isOutput=False)
    dinv_in = nc.declare_dram_parameter("dinv_blk", [128, NBLK], F32, isOutput=False)
    w_in = {}
    for name, shp in [("W1", [IN_F, H]), ("W2", [H, H]), ("W3", [H, H]),
                      ("Wl", [H, C_OUT]), ("B1", [128, H]), ("B2", [128, H]),
                      ("B3", [128, H]), ("BL", [128, C_OUT])]:
        w_in[name] = nc.declare_dram_parameter(name, shp, F32, isOutput=False)
    out_dram = nc.declare_dram_parameter("out_s", [SHP, C_OUT], F32, isOutput=True)

    u_shard = [nc.dram_tensor(f"u_shard{l}", [SHP, H], F32) for l in range(3)]
    u_table = [nc.dram_tensor(f"u_table{l}", [TBL, H], F32) for l in range(3)]

    W_next = {0: "W2", 1: "W3"}
    B_of = {0: "B1", 1: "B2", 2: "B3"}

    with tile.TileContext(nc) as tc, ExitStack() as ctx:
        const = ctx.enter_context(tc.tile_pool(name="const", bufs=1))
        widep = ctx.enter_context(tc.tile_pool(name="widep", bufs=3))
        work = ctx.enter_context(tc.tile_pool(name="work", bufs=3))
        outp = ctx.enter_context(tc.tile_pool(name="outp", bufs=3))
        psum = ctx.enter_context(tc.tile_pool(name="psum", bufs=2, space="PSUM"))

        # ---- load constants ----
        xT = const.tile([IN_F, SHP], F32)
        nc.sync.dma_start(xT[:], xT_in[:])
        nbr = const.tile([128, max(sumd, 1)], I32)
        nc.sync.dma_start(nbr[:], nbr_in[:])
        dinv = const.tile([128, NBLK], F32)
        nc.sync.dma_start(dinv[:], dinv_in[:])
        wt = {}
        for name, shp in [("W1", [IN_F, H]), ("W2", [H, H]), ("W3", [H, H]),
                          ("Wl", [H, C_OUT]), ("B1", [128, H]), ("B2", [128, H]),
                          ("B3", [128, H]), ("BL", [128, C_OUT])]:
            t = const.tile(shp, F32, tag=name)
            nc.sync.dma_start(t[:], w_in[name][:])
            wt[name] = t
        ident = const.tile([128, 128], F32)
        make_identity(nc, ident[:])
        zt = const.tile([128, H], F32)
        nc.vector.memset(zt[:], 0.0)

        # ---- layer-1 table: u1 = dinv * (x @ W1), per block ----
        for b in range(NBLK):
            vP = psum.tile([128, H], F32, tag="vP")
            nc.tensor.matmul(vP[:], lhsT=xT[:, b * 128:(b + 1) * 128],
                             rhs=wt["W1"][:], start=True, stop=True)
            ub = work.tile([128, H], F32, tag="ub")
            nc.vector.tensor_scalar(ub[:], vP[:], dinv[:, b:b + 1], None,
                                    op0=mybir.AluOpType.mult)
            nc.sync.dma_start(u_shard[0][b * 128:(b + 1) * 128, :], ub[:])

        for l in range(3):
            nc.gpsimd.collective_compute(
                "AllGather", mybir.AluOpType.bypass,
                replica_groups=[list(range(CORES))],
                ins=[u_shard[l][:]], outs=[u_table[l][:]],
            )
            for b in range(NBLK):
                d = D[b]
                wide = widep.tile([128, d + 1, H], F32, tag="wide")
                # self-loop column from own shard (contiguous rows)
                nc.sync.dma_start(wide[:, 0, :],
                                  u_shard[l][b * 128:(b + 1) * 128, :])
                for j in range(d):
                    col = offs[b] + j
                    nc.gpsimd.indirect_dma_start(
                        out=wide[:, 1 + j, :], out_offset=None,
                        in_=u_table[l][:],
                        in_offset=bass.IndirectOffsetOnAxis(
                            ap=nbr[:, col:col + 1], axis=0))
                s = work.tile([128, H], F32, tag="s")
                nc.vector.tensor_reduce(
                    s[:], wide[:].rearrange("p j d -> p d j"),
                    axis=mybir.AxisListType.X, op=mybir.AluOpType.add)
                # h = leaky(dinv*s + b)
                t1 = work.tile([128, H], F32, tag="t1")
                nc.vector.tensor_scalar(t1[:], s[:], dinv[:, b:b + 1], None,
                                        op0=mybir.AluOpType.mult)
                t2 = work.tile([128, H], F32, tag="t2")
                nc.vector.tensor_tensor(t2[:], t1[:], wt[B_of[l]][:],
                                        op=mybir.AluOpType.add)
                t3 = work.tile([128, H], F32, tag="t3")
                nc.vector.tensor_scalar(t3[:], t2[:], SLOPE, None,
                                        op0=mybir.AluOpType.mult)
                h = work.tile([128, H], F32, tag="h")
                nc.vector.tensor_tensor(h[:], t2[:], t3[:],
                                        op=mybir.AluOpType.max)
                if l < 2:
                    hs = work.tile([128, H], F32, tag="hs")
                    nc.vector.tensor_scalar(hs[:], h[:], dinv[:, b:b + 1], None,
                                            op0=mybir.AluOpType.mult)
                    trP = psum.tile([H, 128], F32, tag="trP")
                    nc.tensor.transpose(trP[:], hs[:], ident[:])
                    hsT = work.tile([H, 128], F32, tag="hsT")
                    nc.vector.tensor_copy(hsT[:], trP[:])
                    vP = psum.tile([128, H], F32, tag="vP")
                    nc.tensor.matmul(vP[:], lhsT=hsT[:], rhs=wt[W_next[l]][:],
                                     start=True, stop=True)
                    ub = work.tile([128, H], F32, tag="ub")
                    nc.vector.tensor_copy(ub[:], vP[:])
                    if b == NBLK - 1 and SH < SHP:
                        nreal = SH - (NBLK - 1) * 128
                        nc.sync.dma_start(
                            u_shard[l + 1][b * 128:b * 128 + nreal, :],
                            ub[:nreal, :])
                        nc.sync.dma_start(
                            u_shard[l + 1][SH:SHP, :], zt[:SHP - SH, :])
                    else:
                        nc.sync.dma_start(
                            u_shard[l + 1][b * 128:(b + 1) * 128, :], ub[:])
                else:
                    trP = psum.tile([H, 128], F32, tag="trP")
                    nc.tensor.transpose(trP[:], h[:], ident[:])
                    hT = work.tile([H, 128], F32, tag="hsT")
                    nc.vector.tensor_copy(hT[:], trP[:])
                    oP = psum.tile([128, C_OUT], F32, tag="oP")
                    nc.tensor.matmul(oP[:], lhsT=hT[:], rhs=wt["Wl"][:],
                                     start=True, stop=True)
                    o = outp.tile([128, C_OUT], F32, tag="o")
                    nc.vector.tensor_tensor(o[:], oP[:], wt["BL"][:],
                                            op=mybir.AluOpType.add)
                    nc.sync.dma_start(out_dram[b * 128:(b + 1) * 128, :], o[:])
    nc.compile()
    return nc


def _ensure_ntff_hook():
    """The agent image's antenv lacks axon_hooks; shim it and register the
    ctypes NTFF profiling hook so trace=True works under axon."""
    import sys as _sys
    import types
    try:
        import antenv.axon_hooks  # noqa: F401
        return
    except ImportError:
        pass
    mod = types.ModuleType("antenv.axon_hooks")
    _h = [None]
    mod.set_axon_ntff_profile_hook = lambda hook: _h.__setitem__(0, hook)
    mod.get_axon_ntff_profile_hook = lambda: _h[0]
    _sys.modules["antenv.axon_hooks"] = mod
    try:
        from trn_agent_boot.trn_boot import _ntff_profile_via_ctypes
        hook = _ntff_profile_via_ctypes("/opt/axon/libaxon_pjrt.so")
        if hook is not None:
            mod.set_axon_ntff_profile_hook(hook)
    except Exception:
        pass


def kernel(x, edge_index, W1, b1, W2, b2, W3, b3, Wl, bl):
    global LAST_RESULTS
    x = np.asarray(x, dtype=np.float32)
    xTs, nbrs, dinv_blks, D, offs, poss = _host_prep(x, edge_index)
    sumd = offs[-1]

    nc = _build_bass(D, offs, sumd)

    shared = {
        "W1": np.asarray(W1, np.float32), "W2": np.asarray(W2, np.float32),
        "W3": np.asarray(W3, np.float32), "Wl": np.asarray(Wl, np.float32),
        "B1": np.tile(np.asarray(b1, np.float32), (128, 1)),
        "B2": np.tile(np.asarray(b2, np.float32), (128, 1)),
        "B3": np.tile(np.asarray(b3, np.float32), (128, 1)),
        "BL": np.tile(np.asarray(bl, np.float32), (128, 1)),
    }
    in_maps = []
    for c in range(CORES):
        m = dict(shared)
        m["xT"] = xTs[c]
        m["nbr"] = nbrs[c] if sumd else np.zeros((128, 1), np.int32)
        m["dinv_blk"] = dinv_blks[c]
        in_maps.append(m)

    trace = bool(int(os.environ.get("GCN_TRACE", "0")))
    if trace:
        _ensure_ntff_hook()
    res = run_bass_kernel_spmd(nc, in_maps, list(range(CORES)), trace=trace)
    LAST_RESULTS = res

    out = np.empty((N, C_OUT), dtype=np.float32)
    for c in range(CORES):
        shard = res.results[c]["out_s"]
        out[c * SH:(c + 1) * SH] = shard[poss[c]]
    return out
